# revision 20
# baseline (speedup 1.0000x reference)
"""Trainium2 Bass kernel for the scan-RNN problem (B=2048, T=512, H=256).

Data-parallel over batch: 8 cores x 256 rows each. Each core runs NCH
independent software-pipelined chains of RW=256/NCH rows; the T=512
recurrence is serial per chain, and on this hardware a cross-engine
dependency hop costs ~1us, so throughput comes from interleaving many
independent chains over the per-chain latency, not from making single
ops faster.

Math (per step, after host-side folding of gamma/beta into W_update/W_out):
    z   = (h + tanh(x_t*W_embed + b_embed)) @ W' + b'
    u   = tanh(z)
    h   = (u - mean(u)) * rsqrt(var(u) + eps)
x values are integers 0..9, so tanh(x*W_embed+b_embed) is a 10-row table E;
inp @ W' = onehot(x) @ (E @ W'), a K=11 matmul with a host-precomputed
one-hot (row 10 = ones carries the bias b').

Step 0 (h=0) depends only on x_0, so the normalized step-0 state is a host
10-row table H0; the device loop starts at t=1 from hT = H0.T @ onehot(x_0),
materialized directly in transposed layout by PE matmuls. This keeps
var(u) inside [0.33, 0.57] for every device-computed layernorm, so rsqrt is
a 3-op cubic Horner polynomial (2.7e-4 relative) - no tables, no bit tricks.

Per chain and step:
    PE : 2 fp32r matmuls accumulate z (the K=11 onehot@G matmul for step
         t+1 is hoisted into step t - it only needs the DMA'd one-hot);
         2 transpose-mode matmuls re-transpose the new state
    ACT: u = tanh(z)
    DVE: bn_stats/bn_aggr hardware mean+var, cubic rsqrt, fused apply
    ACT/DVE: PSUM evacuation back to SBUF state (alternating per chain)
Chains share PSUM banks in pairs (PE writes disjoint partition halves).
fp32r inputs run the PE at 1 cycle/row (vs 4 for fp32); all fp32r matmul
operands are produced with fp32r-rounding as the compiler requires.

All constants live in one host-packed blob loaded by a single DMA.
"""

import numpy as np

H = 256
EPS = 1e-5
NCORES = 8
NV = 10  # x values are 0..9
KAUG = NV + 1  # + ones row for the bias
OHB = 16  # one-hot steps per DMA batch
NCH = 4  # independent chains per core

# rsqrt(v) ~= c0 + c1 v + c2 v^2 + c3 v^3; relative-minimax fit on
# v in [0.32, 0.58] (max rel err 2.7e-4)
RS_C0 = 3.3361585
RS_C1 = -7.6737317
RS_C2 = 10.436581
RS_C3 = -5.5534572

# blob column layout (all fp32 bit patterns, 128 partitions)
_WP0 = 0           # W' chunk 0 lhsT [128, 256]
_WP1 = 256         # W' chunk 1 lhsT [128, 256]
_ID = 512          # identity [128, 128]
_GA = 640          # G_aug [11(part), 256]
_H0 = 896          # step-0 state table [10(part), 256]
_WO = 1152         # W_out' chunks [128, 2, 16]
_BO = 1184         # row 0: b_out' [1, 16]
_ONES = 1200       # ones [1, 128] on partition 0
_CW = 1200 + 128   # blob width


def build_nc(T, B_local, nch=NCH):
    """Build the Bass program for one core (SPMD: all cores identical)."""
    import concourse.bass as bass
    import concourse.mybir as mybir
    import concourse.tile as tile
    from concourse import bacc

    dt = mybir.dt
    AF = mybir.ActivationFunctionType
    OP = mybir.AluOpType
    nc = bacc.Bacc(None, target_bir_lowering=False, debug=False)

    RW = B_local // nch          # rows per chain
    PR = 128 // RW               # chains per PSUM pair-tile
    NPAIR = nch // PR
    assert RW * nch == B_local and PR * RW == 128
    assert T >= 2

    bf16 = dt.bfloat16

    # ---- DRAM parameters -------------------------------------------------
    ohb = min(OHB, T)
    oh = nc.declare_dram_parameter(
        "oh", [(T + ohb - 1) // ohb, KAUG, ohb * B_local], bf16,
        isOutput=False)
    cst = nc.declare_dram_parameter("cst", [128, _CW], bf16,
                                    isOutput=False)
    out = nc.declare_dram_parameter("out", [B_local, 16], dt.float32,
                                    isOutput=True)

    with tile.TileContext(nc) as tc:
        with (
            tc.tile_pool(name="singles", bufs=1) as singles,
            tc.tile_pool(name="ohpool", bufs=4) as ohpool,
            tc.tile_pool(name="state", bufs=2) as state,
            tc.tile_pool(name="work", bufs=3) as work,
            tc.tile_pool(name="stats", bufs=6) as stats,
            tc.tile_pool(name="psum_z", bufs=1, space="PSUM") as psum_z,
            tc.tile_pool(name="psum_t", bufs=1, space="PSUM") as psum_t,
            tc.tile_pool(name="psum_i", bufs=1, space="PSUM") as psum_i,
        ):
            # ---- one DMA for every constant -----------------------------
            blob = singles.tile([128, _CW], bf16, tag="blob")
            nc.sync.dma_start(out=blob, in_=cst[:, :])
            wp0 = blob[:, _WP0:_WP0 + H]
            wp1 = blob[:, _WP1:_WP1 + H]
            ident = blob[:, _ID:_ID + 128]
            ga_sb = blob[:KAUG, _GA:_GA + H]
            h0_sb = blob[:NV, _H0:_H0 + H]
            wo_sb = blob[:, _WO:_WO + 32].rearrange("p (c h) -> p c h", c=2)
            bo_sb = blob[:1, _BO:_BO + 16]
            ones_row = blob[:1, _ONES:_ONES + 128]

            def new_oh(bi):
                t_ = ohpool.tile([KAUG, ohb, B_local], bf16, tag="oh")
                nc.sync.dma_start(
                    out=t_,
                    in_=oh[bi, :, :].rearrange("v (s b) -> v s b", s=ohb),
                )
                return t_

            oh_bt = new_oh(0)

            def evac(c, dst, src):
                """PSUM -> SBUF state copy, alternating ACT/DVE per chain."""
                if c % 2 == 0:
                    nc.scalar.copy(out=dst, in_=src)
                else:
                    nc.vector.tensor_copy(out=dst, in_=src)

            def prow(c):
                return slice((c % PR) * RW, (c % PR + 1) * RW)

            # ---- t=0: hT = H0.T @ onehot(x_0), straight into state ------
            hTs = [None] * nch
            for c in range(nch):
                bs = bass.ts(c, RW)
                pti = psum_i.tile([128, 2, RW], dt.float32, tag="init")
                for k in range(2):
                    nc.tensor.matmul(
                        pti[:, k, :],
                        lhsT=h0_sb[:, bass.ts(k, 128)],
                        rhs=oh_bt[:NV, 0, bs],
                        start=True, stop=True,
                    )
                hT = state.tile([128, 2, RW], f32r, tag=f"hT{c}")
                evac(c, hT, pti)
                hTs[c] = hT

            # ---- double-step PSUM banks + pipelined G matmuls for t=1 ---
            pzs = []
            for c in range(nch):
                pz2 = psum_z.tile([RW, 2, H], dt.float32, tag=f"pz{c}")
                pzs.append(pz2)
                nc.tensor.matmul(
                    pz2[:, 1, :],
                    lhsT=oh_bt[:, 1 % ohb, bass.ts(c, RW)],
                    rhs=ga_sb, start=True, stop=False,
                )

            jt = psum_i.tile([128, 2, RW], dt.float32, tag="init")
            pts = [None] * NPAIR
            for t in range(1, T):
                if t + 1 < T and (t + 1) % ohb == 0:
                    oh_bt = new_oh((t + 1) // ohb)

                for c in range(nch):
                    bs = bass.ts(c, RW)
                    pz = pzs[c][:, t % 2, :]
                    # ---- PE: recurrent matmuls --------------------------
                    nc.tensor.matmul(
                        pz, lhsT=hTs[c][:, 0, :], rhs=wp0,
                        start=False, stop=False,
                    )
                    nc.tensor.matmul(
                        pz, lhsT=hTs[c][:, 1, :], rhs=wp1,
                        start=False, stop=True,
                    )
                    # ---- ACT: tanh --------------------------------------
                    u = work.tile([RW, H], dt.float32, tag=f"u{c}")
                    nc.scalar.activation(u, pz, AF.Tanh)
                    # ---- DVE: hw mean/var + cubic rsqrt + apply ---------
                    st6 = stats.tile([RW, 6], dt.float32, tag=f"st6{c}")
                    mv = stats.tile([RW, 2], dt.float32, tag=f"mv{c}")
                    nc.vector.bn_stats(st6, u)
                    nc.vector.bn_aggr(mv, st6)
                    mean = mv[:, 0:1]
                    var = mv[:, 1:2]
                    sc = stats.tile([RW, 3], dt.float32, tag=f"sc{c}")
                    t1 = sc[:, 0:1]
                    t2 = sc[:, 1:2]
                    rstd = sc[:, 2:3]
                    nc.vector.tensor_scalar(
                        out=t1, in0=var, scalar1=RS_C3, scalar2=RS_C2,
                        op0=OP.mult, op1=OP.add,
                    )
                    nc.vector.tensor_scalar(
                        out=t2, in0=t1, scalar1=var, scalar2=RS_C1,
                        op0=OP.mult, op1=OP.add,
                    )
                    nc.vector.tensor_scalar(
                        out=rstd, in0=t2, scalar1=var, scalar2=RS_C0,
                        op0=OP.mult, op1=OP.add,
                    )
                    hn = work.tile([RW, H], f32r, tag=f"hn{c}")
                    nc.vector.tensor_scalar(
                        out=hn, in0=u, scalar1=mean, scalar2=rstd,
                        op0=OP.subtract, op1=OP.mult,
                    )
                    nc.tensor.matmul(jt[0:16, 0, 0:16], lhsT=u[:, 0:16],
                                     rhs=ident[:RW, 0:16].bitcast(dt.float32),
                                     start=True, stop=True)
                    # ---- PE: next step's onehot matmul (hoisted) --------
                    if t + 1 < T:
                        nc.tensor.matmul(
                            pzs[c][:, (t + 1) % 2, :],
                            lhsT=oh_bt[:, (t + 1) % ohb, bs],
                            rhs=ga_sb, start=True, stop=False,
                        )
                    # ---- PE transpose + evacuate ------------------------
                    if c % PR == 0:
                        ptp = psum_t.tile([128, 2, PR * RW], f32r,
                                          tag=f"pt{c // PR}")
                        pts[c // PR] = ptp
                    pt = pts[c // PR]
                    for k in range(2):
                        nc.tensor.transpose(
                            pt[:, k, prow(c)], hn[:, bass.ts(k, 128)],
                            ident[:RW, :RW],
                        )
                    hT = state.tile([128, 2, RW], f32r, tag=f"hT{c}")
                    evac(c, hT, pt[:, :, prow(c)])
                    hTs[c] = hT

            # ---- final projection: out = h @ Wout' + bout' --------------
            for c in range(nch):
                po = psum_i.tile([RW, 16], dt.float32, tag="po")
                nc.tensor.matmul(
                    po, lhsT=hTs[c][:, 0, :],
                    rhs=wo_sb[:, 0, :], start=True, stop=False,
                )
                nc.tensor.matmul(
                    po, lhsT=hTs[c][:, 1, :],
                    rhs=wo_sb[:, 1, :], start=False, stop=False,
                )
                nc.tensor.matmul(
                    po, lhsT=ones_row[:, :RW], rhs=bo_sb,
                    start=False, stop=True,
                )
                ot = work.tile([RW, 16], dt.float32, tag=f"ot{c}")
                nc.vector.tensor_copy(out=ot, in_=po)
                nc.sync.dma_start(
                    out=out[c * RW:(c + 1) * RW, :], in_=ot
                )

    nc.finalize()
    return nc


def _prepare_host(x, W_embed, b_embed, W_update, b_update, gamma, beta,
                  W_out, b_out):
    """Fold gamma/beta into the weights; build one-hot + the consts blob."""
    Wp = (gamma[:, None] * W_update).astype(np.float32)  # [H, H]
    bp = (b_update + beta @ W_update).astype(np.float32)  # [H]
    Wo = (gamma[:, None] * W_out).astype(np.float32)  # [H, 10]
    bo = (b_out + beta @ W_out).astype(np.float32)  # [10]

    vals = np.arange(NV, dtype=np.float32)[:, None]
    E = np.tanh(vals @ W_embed + b_embed).astype(np.float32)  # [10, H]
    G = (E @ Wp).astype(np.float32)
    G_aug = np.concatenate([G, bp[None, :]], axis=0)  # [KAUG, H]

    # step-0 table: normalized state after step 0 for each x value
    u0 = np.tanh(G + bp[None, :])  # [10, H]
    mu0 = u0.mean(-1, keepdims=True)
    var0 = ((u0 - mu0) ** 2).mean(-1, keepdims=True)
    H0 = ((u0 - mu0) / np.sqrt(var0 + EPS)).astype(np.float32)

    xi = x[:, :, 0].astype(np.int32)  # [B, T]
    B, T = xi.shape
    oh = np.zeros((T, KAUG, B), np.float32)
    tidx = np.broadcast_to(np.arange(T)[:, None], (T, B))
    bidx = np.broadcast_to(np.arange(B)[None, :], (T, B))
    oh[tidx, xi.T, bidx] = 1.0
    oh[:, NV, :] = 1.0

    cst = np.zeros((128, _CW), np.float32)
    cst[:, _WP0:_WP0 + H] = Wp[0:128]
    cst[:, _WP1:_WP1 + H] = Wp[128:256]
    cst[:, _ID:_ID + 128] = np.eye(128, dtype=np.float32)
    cst[:KAUG, _GA:_GA + H] = G_aug
    cst[:NV, _H0:_H0 + H] = H0
    cst[:, _WO:_WO + 16] = np.pad(Wo[0:128], ((0, 0), (0, 6)))
    cst[:, _WO + 16:_WO + 32] = np.pad(Wo[128:256], ((0, 0), (0, 6)))
    cst[0, _BO:_BO + 10] = bo
    cst[0, _ONES:_ONES + 128] = 1.0
    return oh, cst


def prepare(x, W_embed, b_embed, W_update, b_update, gamma, beta, W_out, b_out,
            T_override=None, B_override=None):
    x = np.asarray(x, np.float32)
    B = x.shape[0] if B_override is None else B_override
    T = x.shape[1] if T_override is None else T_override
    x = x[:B, :T]

    oh, cst = _prepare_host(
        np.asarray(x), np.asarray(W_embed), np.asarray(b_embed),
        np.asarray(W_update), np.asarray(b_update), np.asarray(gamma),
        np.asarray(beta), np.asarray(W_out), np.asarray(b_out),
    )

    B_local = B // NCORES
    nc = build_nc(T, B_local)

    ohb = min(OHB, T)
    in_maps = []
    for c in range(NCORES):
        sl = slice(c * B_local, (c + 1) * B_local)
        ohc = oh[:, :, sl]  # [T, KAUG, B_local]
        ohc = ohc.reshape(T // ohb, ohb, KAUG, B_local).transpose(0, 2, 1, 3)
        ohc = ohc.reshape(T // ohb, KAUG, ohb * B_local)
        in_maps.append({
            "oh": np.ascontiguousarray(ohc),
            "cst": cst,
        })
    return nc, in_maps


def _numpy_fallback(x, W_embed, b_embed, W_update, b_update, gamma, beta,
                    W_out, b_out):
    """Reference math on host; only for inputs the device kernel can't take
    (non-integer x or values outside 0..9 - never happens with the spec'd
    randint fill, but better safe than crashed)."""
    xb = x[:, :, 0]
    B, T = xb.shape
    h = np.zeros((B, H), np.float32)
    for t in range(T):
        inp = np.tanh(xb[:, t:t + 1] @ W_embed + b_embed)
        z = (inp + h) @ W_update + b_update
        u = np.tanh(z)
        mu = u.mean(-1, keepdims=True)
        var = ((u - mu) ** 2).mean(-1, keepdims=True)
        h = (u - mu) / np.sqrt(var + EPS) * gamma + beta
    return (h @ W_out + b_out).astype(np.float32)


def kernel(x, W_embed, b_embed, W_update, b_update, gamma, beta, W_out, b_out,
           T_override=None, B_override=None):
    x = np.asarray(x, np.float32)
    xi = x[:, :, 0]
    if not (np.all(xi == np.round(xi)) and xi.min() >= 0 and xi.max() < NV
            and x.shape[0] % (NCORES * 128) == 0 and x.shape[1] >= 2):
        return _numpy_fallback(
            x, np.asarray(W_embed, np.float32), np.asarray(b_embed, np.float32),
            np.asarray(W_update, np.float32), np.asarray(b_update, np.float32),
            np.asarray(gamma, np.float32), np.asarray(beta, np.float32),
            np.asarray(W_out, np.float32), np.asarray(b_out, np.float32))

    nc, in_maps = prepare(x, W_embed, b_embed, W_update, b_update, gamma, beta,
                          W_out, b_out, T_override, B_override)

    from concourse.bass_utils import run_bass_kernel_spmd

    res = run_bass_kernel_spmd(nc, in_maps, list(range(NCORES)))
    global LAST_RESULT
    LAST_RESULT = res
    outs = [res.results[c]["out"][:, :10] for c in range(NCORES)]
    return np.concatenate(outs, axis=0).astype(np.float32)


LAST_RESULT = None
